# revision 1
# baseline (speedup 1.0000x reference)
"""AdaptiveAttention on 8 TRN2 NeuronCores.

Model (b=2, n=2048, dim=1024, 16 heads x 64, NUM_W=4 adaptive weights):
    gates = softmax(x @ Wg)                                  # [b, n, 4]
    qkv_w = x @ Wqkv  (packed (qkv, h, d, w))                # per w: q,k,v
    q,k,v = sum_w gates_w * qkv_w                            # gated combine
    out   = softmax(q k^T / sqrt(64)) v                      # per head
    y     = sum_w gates_w * (out @ Wout_w)                   # gated out-proj

Sharding: core c -> batch beta=c//4, head-group g=c%4 (4 heads each).
Each core computes a partial out-projection over its 256 dim_inner rows;
the host sums the 4 partials per batch (partial-sum output sharding).

Key kernel ideas:
 - Single transpose of x -> xT [dim, tok]; all matmuls then run natively.
 - Gated combine folded into the QKV matmul: accumulate over (dim-chunk, w)
   of Wqkv_w^T @ (xT * G_w) in PSUM - the w-combine is free on TensorE.
 - Scores computed transposed, ST = kT_tile^T qT -> [keys, q]; exp(ST)
   tiles are directly the lhsT for the PV matmul. Row-sums for softmax come
   free from a ones-column appended to v (stationary M=65).
 - bf16 on the PE moving/stationary side, f32 accumulation in PSUM.
"""

import os

import numpy as np

P = 128
N_TOK = 2048          # tokens per batch
DIM = 1024
DH = 64               # head dim
W = 4                 # adaptive weights
NH_LOC = 4            # heads per core
FEAT = NH_LOC * DH    # 256 local dim_inner feats
CB = DIM // P         # 8 dim chunks
TB = 4                # token blocks (512 each)
TBS = N_TOK // TB     # 512
KC = N_TOK // P       # 16 key chunks
QB = 4                # query blocks (512 each)
QBS = N_TOK // QB     # 512
VW = DH + 1           # 65: v columns + ones column for row-sums

_BUILT = None


def _split_waits(nc, keep=1):
    """Workaround: this neuronxcc walrus build rejects instructions carrying
    more than one sync wait ("Too many sync wait commands") on several codegen
    paths (Drain, CollectiveCompute, PSEUDO_DMA_DIRECT2D, ...). Hoist excess
    waits onto dedicated single-wait EventSemaphore carriers inserted
    immediately before the instruction on the same engine."""
    import concourse.mybir as mybir

    for fn in nc.m.functions:
        for bb in fn.blocks:
            new_list = []
            for inst in bb.instructions:
                si = inst.sync_info
                waits = list(si.on_wait) if si is not None else []
                if len(waits) > keep:
                    extra, kept = waits[keep:], waits[:keep]
                    for j, w in enumerate(extra):
                        c = mybir.InstEventSemaphore(
                            name=f"{inst.name}-pw{j}", ins=[], outs=[],
                            sync_info=mybir.SyncInfo(on_wait=[w], on_update=[]),
                        )
                        c.engine = inst.engine
                        new_list.append(c)
                    si.on_wait.clear()
                    for w in kept:
                        si.on_wait.append(w)
                new_list.append(inst)
            bb.instructions[:] = new_list




def _patch_tile_exit():
    """Trim the TileContext exit: split the drain's waits (walrus single-wait
    limit) and drop the final all-engine barrier - after the first barrier no
    engine has further instructions, so only the semaphore clears remain."""
    import concourse.tile as tile
    from concourse.vector_clock import ScopedClock
    if getattr(tile.TileContext, "_exit_trimmed", False):
        return

    def _drain_and_barrier(self, tick_clock, wait_clock):
        nc = self.nc
        probe = nc.sync.nop()
        wait_clock.add_sem_waits(probe.ins, ScopedClock({None: tick_clock.global_clock}))
        si = probe.ins.sync_info
        waits = list(si.on_wait) if si is not None else []
        if si is not None:
            si.on_wait.clear()
        handles = {h.name: h for h in self.sems.allocated().values()}
        for w in waits:
            h = handles.get(w.ant_name)
            assert h is not None, f"no semaphore handle named {w.ant_name}"
            nc.sync.wait_ge(h, w.wait_value)
        nc.sync.drain()
        nc.all_engine_barrier()
        assert self.sems is not None
        popped = nc._tile_sem_poison_stack.pop()
        assert popped is self._sem_poison
        nc.clear_and_free_semaphores(list(self.sems.allocated().values()))

    tile.TileContext._drain_and_barrier = _drain_and_barrier
    tile.TileContext._exit_trimmed = True


def _build():
    import concourse.bass as bass
    import concourse.mybir as mybir
    import concourse.tile as tile
    from concourse.masks import make_identity

    F32 = mybir.dt.float32
    BF16 = mybir.dt.bfloat16
    EXP = mybir.ActivationFunctionType.Exp
    MUL = mybir.AluOpType.mult

    _patch_tile_exit()
    nc = bass.Bass()
    x_ext = nc.declare_dram_parameter("x", [N_TOK, DIM], BF16, isOutput=False)
    wqkv_ext = nc.declare_dram_parameter("wqkv", [DIM, 3 * FEAT * W], BF16, isOutput=False)
    wg_ext = nc.declare_dram_parameter("wg", [DIM, W], BF16, isOutput=False)
    wout_ext = nc.declare_dram_parameter("wout", [FEAT, DIM * W], BF16, isOutput=False)
    out_ext = nc.declare_dram_parameter("out", [N_TOK, DIM], BF16, isOutput=True)

    with tile.TileContext(nc) as tc:
        with (
            tc.tile_pool(name="const", bufs=1) as constp,
            tc.tile_pool(name="big", bufs=1) as bigp,
        ):
            ident = constp.tile([P, P], F32, tag="idf", name="idf")
            make_identity(nc, ident[:])
            ident_bf = constp.tile([P, P], BF16, tag="idb", name="idb")
            nc.vector.tensor_copy(ident_bf[:], ident[:])
            ones_row = constp.tile([1, P], F32, tag="ones", name="ones")
            nc.vector.memset(ones_row[:], 1.0)
            ones_bf = constp.tile([1, P], BF16, tag="onesb", name="onesb")
            nc.vector.memset(ones_bf[:], 1.0)
            # bf16 selector (row 0 ones) for the row-sum broadcasts
            sel0b = constp.tile([P, P], BF16, tag="sel0b", name="sel0b")
            nc.gpsimd.memset(sel0b[:], 0.0)
            nc.gpsimd.affine_select(
                out=sel0b[:], in_=sel0b[:],
                compare_op=mybir.AluOpType.not_equal, fill=1.0,
                base=0, pattern=[[0, P]], channel_multiplier=1)
            # sel[w] [4, 128] bf16: row w ones, others zero (G-broadcast lhsT)
            sels = []
            for w in range(W):
                s = constp.tile([P, P], BF16, tag=f"sel{w}", name=f"sel{w}")
                nc.gpsimd.memset(s[:], 0.0)
                nc.gpsimd.affine_select(
                    out=s[:], in_=s[:],
                    compare_op=mybir.AluOpType.not_equal, fill=1.0,
                    base=-w, pattern=[[0, P]], channel_multiplier=1)
                sels.append(s)

            # resident stage outputs
            G = [bigp.tile([P, N_TOK], BF16, tag=f"G{w}", name=f"G{w}") for w in range(W)]
            qTA = [bigp.tile([P, N_TOK], BF16, tag=f"qTa{i}", name=f"qTa{i}") for i in range(2)]
            qTB = [bigp.tile([P, N_TOK], BF16, tag=f"qTb{i}", name=f"qTb{i}") for i in range(2)]
            for i in range(2):
                nc.vector.memset(qTA[i][DH:P, :], 0.0)
                nc.vector.memset(qTB[i][0:DH, :], 0.0)
            kT = [bigp.tile([P, N_TOK], BF16, tag=f"kT{i}", name=f"kT{i}") for i in range(2)]
            v_ext = [bigp.tile([P, NH_LOC * VW], BF16, tag=f"vx{kc}", name=f"vx{kc}") for kc in range(KC)]
            outT = [bigp.tile([P, N_TOK], BF16, tag=f"oT{i}", name=f"oT{i}") for i in range(2)]

            for kc in range(KC):
                ve = v_ext[kc].rearrange("p (h v) -> p h v", h=NH_LOC, v=VW)
                nc.vector.memset(ve[:, :, DH:DH + 1], 1.0)

            # ================= Stage A: xT, gates, QKV =================
            with (
                tc.tile_pool(name="wq", bufs=1) as wqp,
                tc.tile_pool(name="xtp", bufs=1) as xtp,
                tc.tile_pool(name="xin", bufs=3) as xinp,
                tc.tile_pool(name="scrA", bufs=3) as scrp,
                tc.tile_pool(name="yw", bufs=3) as ywp,
                tc.tile_pool(name="ps_qkv", bufs=1, space="PSUM") as ps_qkv,
                tc.tile_pool(name="ps_tr", bufs=2, space="PSUM") as ps_tr,
            ):
                xT = [xtp.tile([P, N_TOK], BF16, tag=f"xT{c}", name=f"xT{c}") for c in range(CB)]
                gT4 = xtp.tile([P, N_TOK], BF16, tag="gT4", name="gT4")
                nc.vector.memset(gT4[:], 0.0)
                # pass 1: xT via hardware X-bar DMA-transpose straight from
                # DRAM (x is bf16). Per token-block so all dim-chunks of block
                # 0 land first (the gates pass needs every chunk); issued
                # before the much larger wqkv prefetch.
                # wg FIRST (tiny; the gates pass blocks on it), then x
                # blocks 0-1 on the SP HW-DGE queue, wqkv behind them, and x
                # blocks 2-3 on the ACT HW-DGE queue - the two DMA streams
                # run in parallel, so the first gates matmul can issue at
                # ~11us instead of ~54us.
                wg_sb = constp.tile([P, CB * W], BF16, tag="wg", name="wg")
                nc.sync.dma_start(
                    wg_sb[:].rearrange("p (c w) -> p c w", c=CB, w=W),
                    wg_ext[:].rearrange("(c p) w -> p c w", p=P))
                for t in range(2):
                    for c in range(CB):
                        nc.sync.dma_start(
                            xT[c][:, t * TBS:(t + 1) * TBS],
                            x_ext[t * TBS:(t + 1) * TBS, c * P:(c + 1) * P],
                            transpose=True)
                # wqkv SBUF layout per dim-chunk: [128, (qkv 3)(h 4)(d 64)(w 4)]
                wqkv_sb = [wqp.tile([P, 3 * FEAT * W], BF16, tag=f"wqkv{c}", name=f"wqkv{c}")
                           for c in range(CB)]
                for c in range(CB):
                    nc.sync.dma_start(wqkv_sb[c][:], wqkv_ext[c * P:(c + 1) * P, :])
                for t in range(2, TB):
                    for c in range(CB):
                        nc.sync.dma_start(
                            xT[c][:, t * TBS:(t + 1) * TBS],
                            x_ext[t * TBS:(t + 1) * TBS, c * P:(c + 1) * P],
                            transpose=True)
                # pass 2a: gates + G broadcast for all blocks
                for t in range(TB):
                    ts = t * TBS
                    # gates (natural layout), then per-w column transposes
                    # (each lands at partition 0 as required downstream)
                    gns = []
                    for tt in range(4):
                        gp = ps_tr.tile([P, TBS], F32, tag="tr", name="tr")
                        for c in range(CB):
                            nc.tensor.matmul(
                                gp[:, 0:W], xT[c][:, ts + tt * P: ts + (tt + 1) * P],
                                wg_sb[:, c * W:(c + 1) * W],
                                start=(c == 0), stop=(c == CB - 1))
                        ge = scrp.tile([P, W], F32, tag="ge", name="ge")
                        gs = scrp.tile([P, 1], F32, tag="gs", name="gs")
                        nc.scalar.activation(ge[:], gp[:, 0:W], EXP, accum_out=gs[:])
                        gr = scrp.tile([P, 1], F32, tag="gr", name="gr")
                        nc.vector.reciprocal(gr[:], gs[:])
                        gn = scrp.tile([P, W], F32, tag=f"gn{tt}", name=f"gn{tt}")
                        nc.vector.tensor_scalar_mul(gn[:], ge[:], gr[:])
                        gns.append(gn)
                    for tt in range(4):
                        gtp = ps_tr.tile([W, TBS], F32, tag="tr", name="gtp")
                        nc.tensor.transpose(
                            gtp[:, 0:P], gns[tt][:], ident[:])
                        nc.vector.tensor_copy(
                            gT4[0:W, ts + tt * P: ts + (tt + 1) * P], gtp[:, 0:P])
                    # broadcast gates across partitions: G_w[p, tok] = g[tok, w]
                    for w in range(W):
                        gb = ps_tr.tile([P, TBS], F32, tag="tr", name="tr")
                        nc.tensor.matmul(
                            gb[:], sels[w][:], gT4[:, ts: ts + TBS],
                            start=True, stop=True)
                        nc.scalar.copy(G[w][:, ts: ts + TBS], gb[:])
                # pass 2b: gated QKV projection
                for t in range(TB):
                    ts = t * TBS
                    # QKV: accumulate over (c, w) of Wqkv_w^T @ (xT_c * G_w)
                    pq = [ps_qkv.tile([P, TBS], F32, tag=f"pq{i}", name=f"pq{i}") for i in range(2)]
                    pk = [ps_qkv.tile([P, TBS], F32, tag=f"pk{i}", name=f"pk{i}") for i in range(2)]
                    pv = [ps_qkv.tile([P, TBS], F32, tag=f"pv{i}", name=f"pv{i}") for i in range(2)]
                    for c in range(CB):
                        for w in range(W):
                            yw = ywp.tile([P, TBS], BF16, tag="yw", name="yw")
                            nc.vector.tensor_tensor(
                                yw[:], xT[c][:, ts: ts + TBS], G[w][:, ts: ts + TBS], MUL)
                            wv = wqkv_sb[c].rearrange(
                                "p (q h d w) -> p q h d w", q=3, h=NH_LOC, d=DH, w=W)
                            st = (c == 0 and w == 0)
                            sp = (c == CB - 1 and w == W - 1)
                            for hp in range(2):
                                nc.tensor.matmul(
                                    pq[hp][:], wv[:, 0, 2 * hp:2 * hp + 2, :, w], yw[:],
                                    start=st, stop=sp)
                                nc.tensor.matmul(
                                    pk[hp][:], wv[:, 1, 2 * hp:2 * hp + 2, :, w], yw[:],
                                    start=st, stop=sp)
                                nc.tensor.matmul(
                                    pv[hp][:], wv[:, 2, 2 * hp:2 * hp + 2, :, w], yw[:],
                                    start=st, stop=sp)
                    vT_sb = [scrp.tile([P, TBS], BF16, tag=f"vT{i}", name=f"vT{i}") for i in range(2)]
                    for hp in range(2):
                        nc.scalar.copy(qTA[hp][0:DH, ts: ts + TBS], pq[hp][0:DH, :])
                        nc.scalar.copy(qTB[hp][DH:P, ts: ts + TBS], pq[hp][DH:P, :])
                        nc.scalar.copy(kT[hp][:, ts: ts + TBS], pk[hp][:])
                        nc.scalar.copy(vT_sb[hp][:], pv[hp][:])
                    # v back to natural layout [keys, (h, d | ones)]
                    for tt in range(4):
                        kc = t * 4 + tt
                        for hp in range(2):
                            vtp = ps_tr.tile([P, TBS], BF16, tag="tr", name="tr")
                            nc.tensor.transpose(
                                vtp[:, 0:P], vT_sb[hp][:, tt * P:(tt + 1) * P],
                                ident_bf[:])
                            dst = v_ext[kc].rearrange(
                                "p (h v) -> p h v", h=NH_LOC, v=VW)
                            src = vtp[:, 0:P].rearrange(
                                "p (h d) -> p h d", h=2, d=DH)
                            nc.scalar.copy(dst[:, 2 * hp:2 * hp + 2, 0:DH], src)

            # ========= Stage B+C: attention fused with out-projection =====
            # Stage C's matmuls for query-block qb-1 are issued between the
            # attention blocks (software pipelining) so the PE never waits on
            # the normalize/gating elementwise chain.
            with (
                tc.tile_pool(name="pt", bufs=2) as ptp,
                tc.tile_pool(name="scrB", bufs=3) as scrbp,
                tc.tile_pool(name="woutp", bufs=1) as woutp,
                tc.tile_pool(name="owp", bufs=2) as owp,
                tc.tile_pool(name="zp", bufs=2) as zp,
                tc.tile_pool(name="ps_st", bufs=2, space="PSUM") as ps_st,
                tc.tile_pool(name="ps_pv", bufs=2, space="PSUM") as ps_pv,
                tc.tile_pool(name="ps_z", bufs=2, space="PSUM") as ps_z,
            ):
                wout_sb = [woutp.tile([P, DIM * W], BF16, tag=f"wo{fc}", name=f"wo{fc}")
                           for fc in range(2)]
                for fc in range(2):
                    nc.sync.dma_start(
                        wout_sb[fc][:], wout_ext[fc * P:(fc + 1) * P, :])
                rs = [scrbp.tile([P, QBS], BF16, tag=f"rs{r}", name=f"rs{r}",
                                 bufs=1)
                      for r in range(16)]
                for r in range(16):
                    nc.vector.memset(rs[r][:], 0.0)
                ow = {}

                def finalize_half(qb, oi):
                    # normalize outT[oi][:, qb] by the softmax row-sums and
                    # apply the output gates for that half (heads 2oi, 2oi+1)
                    qs = qb * QBS
                    rb = ps_st.tile([P, 2 * QBS], F32, tag="st", name="st")
                    nc.tensor.matmul(
                        rb[0:DH, 0:QBS], sel0b[:, 0:DH],
                        rs[qb * 4 + 2 * oi][:], start=True, stop=True)
                    nc.tensor.matmul(
                        rb[DH:P, 0:QBS], sel0b[:, 0:DH],
                        rs[qb * 4 + 2 * oi + 1][:], start=True, stop=True,
                        tile_position=(0, 64))
                    # reciprocal as exp(-ln(x)) on ACT: ~0.9us vs 3.3us for
                    # the DVE reciprocal (row-sums are positive, ~1e-6 rel err)
                    rbc = scrbp.tile([P, QBS], F32, tag="rbc", name="rbc",
                                     bufs=2)
                    nc.scalar.activation(
                        rbc[:], rb[:, 0:QBS], mybir.ActivationFunctionType.Ln)
                    rbs = scrbp.tile([P, QBS], F32, tag="rbs", name="rbs")
                    nc.scalar.activation(rbs[:], rbc[:], EXP, scale=-1.0)
                    sl = outT[oi][:, qs: qs + QBS]
                    nc.vector.tensor_tensor(sl, sl, rbs[:], MUL)
                    for w in range(W):
                        o = owp.tile([P, QBS], BF16, tag=f"ow{oi}{w}",
                                     name=f"ow{oi}{w}")
                        nc.vector.tensor_tensor(
                            o[:], outT[oi][:, qs: qs + QBS],
                            G[w][:, qs: qs + QBS], MUL)
                        ow[(qb, oi, w)] = o

                pending = []

                def zproj_thunks(qb):
                    # out-projection for qb as a flat list of issue thunks so
                    # its matmuls can be interleaved into the (ACT-bound)
                    # attention kc loops as TensorE filler work
                    thunks = []
                    for tt in range(4):
                        box = []

                        def alloc(box=box):
                            box.append([ps_z.tile([P, 512], F32, tag="z",
                                                  name="z")
                                        for _ in range(2)])
                        thunks.append(alloc)
                        for fc in range(2):
                            for w in range(W):
                                for half in range(2):
                                    def mm(box=box, tt=tt, fc=fc, w=w,
                                           half=half, qb=qb):
                                        wv = wout_sb[fc].rearrange(
                                            "p (eh e w) -> p eh w e",
                                            eh=2, e=512, w=W)
                                        nc.tensor.matmul(
                                            box[0][half][:],
                                            ow[(qb, fc, w)][:, tt * P:(tt + 1) * P],
                                            wv[:, half, w, :],
                                            start=(fc == 0 and w == 0),
                                            stop=(fc == 1 and w == W - 1))
                                    thunks.append(mm)

                        def fin(box=box, tt=tt, qb=qb):
                            zps = box.pop()
                            ttk = qb * 4 + tt
                            zs = zp.tile([P, DIM], BF16, tag="zs", name="zs")
                            for half in range(2):
                                nc.vector.tensor_copy(
                                    zs[:, half * 512:(half + 1) * 512],
                                    zps[half][:])
                            nc.sync.dma_start(
                                out_ext[ttk * P:(ttk + 1) * P, :], zs[:])
                        thunks.append(fin)
                    return thunks

                def pump(n):
                    for _ in range(min(n, len(pending))):
                        pending.pop(0)()

                # Head-pair phases, software-pipelined one pair deep: the PV
                # matmuls of pair i-1 are interleaved into pair i's ST loop so
                # the PE stays dense while ACT computes the exps.
                pairs = [(qb, hp) for qb in range(QB) for hp in range(2)]
                prev = None  # (qb, hp, pts, po_tiles)

                def pv_finalize(qb_p, hp_p, pts_p, pos_p):
                    for hh in range(2):
                        h = hp_p * 2 + hh
                        qs_p = qb_p * QBS
                        oi, orow = divmod(h * DH, P)
                        nc.scalar.copy(
                            outT[oi][orow: orow + DH, qs_p: qs_p + QBS],
                            pos_p[hh][0:DH, :])
                        nc.scalar.copy(rs[qb_p * 4 + h][0:1, :], pos_p[hh][DH:VW, :])

                for i, (qb, hp) in enumerate(pairs):
                    qs = qb * QBS
                    pts = ptp.tile([P, KC * 2 * QBS], BF16, tag="pt", name="pt")
                    if prev is not None:
                        qb_p, hp_p, pts_p, _ = prev
                        pos_p = [ps_pv.tile([VW, QBS], F32, tag="po", name="po")
                                 for _ in range(2)]
                        prev = (qb_p, hp_p, pts_p, pos_p)
                    for kc in range(KC):
                        ks = kc * P
                        s2 = ps_st.tile([P, 2 * QBS], F32, tag="st", name="st")
                        nc.tensor.matmul(
                            s2[:, 0:QBS], kT[hp][:, ks: ks + P],
                            qTA[hp][:, qs: qs + QBS],
                            start=True, stop=True)
                        nc.tensor.matmul(
                            s2[:, QBS:2 * QBS], kT[hp][:, ks: ks + P],
                            qTB[hp][:, qs: qs + QBS],
                            start=True, stop=True)
                        if prev is not None:
                            qb_p, hp_p, pts_p, pos_p = prev
                            for hh in range(2):
                                h = hp_p * 2 + hh
                                nc.tensor.matmul(
                                    pos_p[hh][:],
                                    v_ext[kc][:, h * VW:(h + 1) * VW],
                                    pts_p[:, kc * 2 * QBS + hh * QBS:
                                          kc * 2 * QBS + (hh + 1) * QBS],
                                    start=(kc == 0), stop=(kc == KC - 1))
                        nc.scalar.activation(
                            pts[:, kc * 2 * QBS:(kc + 1) * 2 * QBS], s2[:],
                            EXP, scale=0.125)
                        pump(3)
                    if prev is not None:
                        qb_p, hp_p, pts_p, pos_p = prev
                        pv_finalize(qb_p, hp_p, pts_p, pos_p)
                        if hp_p == 1 and qb_p < QB - 1:
                            finalize_half(qb_p, 0)
                            finalize_half(qb_p, 1)
                        if hp_p == 0 and qb_p > 0:
                            # out-projection for the qb finalized one pair ago:
                            # its gated ow tiles have had a full phase to land
                            pending.extend(zproj_thunks(qb_p - 1))
                        if hp_p == 0 and qb_p == QB - 1:
                            # last qb: heads 0-1 can normalize a phase early,
                            # shortening the tail's elementwise chain
                            finalize_half(QB - 1, 0)
                    prev = (qb, hp, pts, None)
                # drain the last pair
                qb_p, hp_p, pts_p, _ = prev
                pos_p = [ps_pv.tile([VW, QBS], F32, tag="po", name="po")
                         for _ in range(2)]
                for kc in range(KC):
                    for hh in range(2):
                        h = hp_p * 2 + hh
                        nc.tensor.matmul(
                            pos_p[hh][:], v_ext[kc][:, h * VW:(h + 1) * VW],
                            pts_p[:, kc * 2 * QBS + hh * QBS:
                                  kc * 2 * QBS + (hh + 1) * QBS],
                            start=(kc == 0), stop=(kc == KC - 1))
                    pump(2)
                pv_finalize(qb_p, hp_p, pts_p, pos_p)
                finalize_half(qb_p, 1)
                pump(len(pending))
                for th in zproj_thunks(qb_p):
                    th()

    _split_waits(nc)
    return nc


def _get_built():
    global _BUILT
    if _BUILT is None:
        _BUILT = _build()
    return _BUILT


def kernel(x, Wqkv, Wg, Wout, mask=None, **_ignored):
    """Full inputs in, full output out. mask is all-ones by construction and
    is ignored (attention over an all-true mask is mask-free)."""
    from concourse.bass_utils import run_bass_kernel_spmd

    import ml_dtypes
    bf16 = ml_dtypes.bfloat16
    x = np.asarray(x, dtype=np.float32).astype(bf16)
    Wqkv = np.asarray(Wqkv, dtype=np.float32).astype(bf16)
    Wg = np.asarray(Wg, dtype=np.float32).astype(bf16)
    Wout = np.asarray(Wout, dtype=np.float32).astype(bf16)
    b = x.shape[0]

    in_maps = []
    for c in range(8):
        beta, g = c // 4, c % 4
        cols = [Wqkv[:, (q * 16 + 4 * g) * 256:(q * 16 + 4 * g + 4) * 256]
                for q in range(3)]
        in_maps.append({
            "x": np.ascontiguousarray(x[beta]),
            "wqkv": np.ascontiguousarray(np.concatenate(cols, axis=1)),
            "wg": np.ascontiguousarray(Wg),
            "wout": np.ascontiguousarray(Wout[g * 256:(g + 1) * 256, :]),
        })

    nc = _get_built()
    trace = bool(int(os.environ.get("KBENCH_TRACE", "0")))
    res = run_bass_kernel_spmd(nc, in_maps, core_ids=list(range(8)), trace=trace)
    kernel.last_exec_time_ns = res.exec_time_ns

    out = np.zeros((b, N_TOK, DIM), dtype=np.float32)
    for c in range(8):
        out[c // 4] += res.results[c]["out"].astype(np.float32)
    return out

